# Initial kernel scaffold
#
"""Int8 per-token-quantized linear (MluQuantLinearInt8) on 8 Trainium2 cores.

  out[s, n] = (sum_k q[s,k] * w[n,k]) * x_scale[s] * w_scale[n]
  q = round(x / x_scale) clipped to [-127, 127],  x_scale = max(|x|_row, 1e-8)/127

Sharding: data-parallel over tokens (512/core); weights replicated, streamed
once per core. Weights are host-pretransposed to [K, N] bf16 (int8 values are
exact in bf16, so int8xint8 products accumulate exactly in fp32 PSUM).
Per-core GEMM is weights-stationary: lhsT = wT[128k, 128n] chunks,
rhs = qT[128k, 512tok], out psum [128n, 512tok]; dequant fused into the
PSUM->SBUF eviction; output stored transposed [N, 512] and re-assembled on
host.
"""

import sys
from contextlib import ExitStack
from functools import lru_cache

import numpy as np

for _p in ("/opt/trn_rl_repo", "/root/.axon_site/_ro/trn_rl_repo"):
    if _p not in sys.path:
        sys.path.append(_p)

import ml_dtypes  # noqa: E402

import concourse.bass as bass  # noqa: E402
import concourse.bass2jax as bass2jax  # noqa: E402
import concourse.mybir as mybir  # noqa: E402
import concourse.tile as tile  # noqa: E402
from concourse.bass_utils import (  # noqa: E402
    compile_bir_kernel as _orig_compile_bir_kernel,
    run_bass_kernel_spmd,
)
from concourse.masks import make_identity  # noqa: E402

# The walrus build in this container accepts only ONE sync-wait per
# instruction ("Too many sync wait commands", CoreV3GenImpl setupSyncWait) —
# Tile's kernel-tail drain carries several. Split extra waits onto preceding
# single-wait EventSemaphore carriers on the same engine (engine program order
# makes the AND of waits equivalent).
import json as _json  # noqa: E402


def _split_multi_waits(bir_json):
    d = _json.loads(bir_json)
    changed = False
    for fn in d.get("functions", []):
        for bb in fn.get("blocks", []) or []:
            insts = bb.get("instructions")
            if not insts:
                continue
            out = []
            for ins in insts:
                si = ins.get("sync_info")
                waits = (si or {}).get("on_wait") or []
                if len(waits) > 1:
                    for j, w in enumerate(waits[:-1]):
                        out.append(
                            {
                                "engine": ins.get("engine"),
                                "ins": [],
                                "outs": [],
                                "name": f"{ins.get('name', 'I')}_w{j}",
                                "opcode": "EventSemaphore",
                                "sync_info": {"on_update": [], "on_wait": [w]},
                            }
                        )
                    si["on_wait"] = [waits[-1]]
                    changed = True
                out.append(ins)
            bb["instructions"] = out
    if not changed:
        return bir_json
    return _json.dumps(d).encode()


def _patched_compile_bir_kernel(bir_json, tmpdir, neff_name="file.neff"):
    return _orig_compile_bir_kernel(
        _split_multi_waits(bir_json), tmpdir, neff_name=neff_name
    )


bass2jax.compile_bir_kernel = _patched_compile_bir_kernel

P = 128
NCORES = 8
S, K_FULL, N_FULL = 4096, 4096, 16384
QMAX = 127.0
MAGIC = 12582912.0  # 1.5 * 2**23: (y + MAGIC) - MAGIC == RNE-round(y) for |y| < 2**22
F32 = mybir.dt.float32
BF16 = mybir.dt.bfloat16


def build_nc(S_C, K, N, NSUB=4, exact_divide=True):
    """One-core program; SPMD-replicated across cores by the runner.

    Inputs (per core):
      x   [S_C, K]  f32 - this core's token slice
      wt  [WC, P, KC, NSUB*P] bf16 - weights, host-packed as SBUF-layout chunks
      ws  [P, NT]   f32 - weight_scale packed ws[p, nt] = weight_scale[nt*128+p]
    Output:
      outT [N, S_C] f32 - dequantized output, transposed
    """
    KC = K // P  # contraction chunks
    TT = S_C // P  # token tiles
    NT = N // P  # output-channel tiles (one psum tile each)
    WC = NT // NSUB  # streamed weight chunks

    nc = bass.Bass()
    x = nc.declare_dram_parameter("x", [S_C, K], F32, isOutput=False)
    wt = nc.declare_dram_parameter("wt", [WC, P, KC, NSUB * P], BF16, isOutput=False)
    ws = nc.declare_dram_parameter("ws", [P, NT], F32, isOutput=False)
    outT = nc.declare_dram_parameter("outT", [N, S_C], F32, isOutput=True)
    xs_scratch = nc.dram_tensor("xs_scratch", [S_C], F32)

    outT_t = outT.rearrange("(nt p) s -> nt p s", p=P)

    with tile.TileContext(nc) as tc, ExitStack() as ctx:
        const_pool = ctx.enter_context(tc.tile_pool(name="const", bufs=1))
        xpool = ctx.enter_context(tc.tile_pool(name="xp", bufs=4))
        qpool = ctx.enter_context(tc.tile_pool(name="qp", bufs=3))
        qt_pool = ctx.enter_context(tc.tile_pool(name="qt", bufs=1))
        wpool = ctx.enter_context(tc.tile_pool(name="wp", bufs=2))
        opool = ctx.enter_context(tc.tile_pool(name="op", bufs=4))
        spool = ctx.enter_context(tc.tile_pool(name="sp", bufs=1))
        pt_pool = ctx.enter_context(tc.tile_pool(name="ptp", bufs=1, space="PSUM"))
        ps_pool = ctx.enter_context(tc.tile_pool(name="psp", bufs=7, space="PSUM"))

        ident_f32 = const_pool.tile([P, P], F32)
        make_identity(nc, ident_f32)

        ws_sb = const_pool.tile([P, NT], F32)
        nc.sync.dma_start(ws_sb, ws[:, :])

        # ---- Phase 1: per-token dynamic int8 quantization + transpose ----
        # qT[k%128, t, k//128, tok%128]: each token tile's DMA-transpose target
        # is per-partition contiguous (non-contiguous dst breaks DMA transpose)
        qT = qt_pool.tile([P, TT, KC, P], BF16)
        xs_all = spool.tile([P, TT], F32)  # xs_all[p, t] = x_scale[t*128+p]

        def load_wchunk(wc):
            wtile = wpool.tile([P, KC, NSUB * P], BF16, tag="wtile")
            half = KC // 2
            nc.sync.dma_start(wtile[:, :half], wt[wc, :, :half])
            nc.sync.dma_start(wtile[:, half:], wt[wc, :, half:])
            return wtile

        # weight chunk 0 FIRST: it must be resident the moment qT completes,
        # and the x loads + XBAR transposes otherwise saturate DMA ahead of it
        # (measured 37.7us PE stall when it queued behind them)
        wtiles = {0: load_wchunk(0)}

        # then all token-tile loads (they pace the quant chain)
        xts = []
        for t in range(TT):
            xt = xpool.tile([P, K], F32)
            nc.sync.dma_start(xt, x[t * P : (t + 1) * P, :])
            xts.append(xt)

        for t in range(TT):
            xt = xts[t]
            amax = spool.tile([P, 1], F32, tag="amax")
            nc.vector.tensor_reduce(
                out=amax,
                in_=xt,
                axis=mybir.AxisListType.X,
                op=mybir.AluOpType.max,
                apply_absolute_value=True,
            )
            # amax' = max(amax, 1e-8); x_scale = amax'/127 (~1ulp, via *1/127);
            # q = round(x * (127 * recip(amax'))) - DVE has no divide, but
            # reciprocal is bit-exact; the ~1ulp quantizer error flips a
            # rounding boundary on ~0.1 elements per 4096-row (negligible).
            nc.vector.tensor_scalar(
                amax, amax, 1e-8, None, op0=mybir.AluOpType.max
            )
            nc.vector.tensor_scalar(
                xs_all[:, t : t + 1],
                amax,
                float(np.float32(1.0 / 127.0)),
                None,
                op0=mybir.AluOpType.mult,
            )
            inv = spool.tile([P, 1], F32, tag="inv")
            nc.vector.reciprocal(inv, amax)
            nc.vector.tensor_scalar(
                inv, inv, QMAX, None, op0=mybir.AluOpType.mult
            )
            # y = x*inv + MAGIC on the (otherwise idle) scalar engine
            nc.scalar.activation(
                xt, xt, mybir.ActivationFunctionType.Copy, bias=MAGIC, scale=inv
            )
            q = qpool.tile([P, K], BF16)
            nc.vector.tensor_scalar(
                q, xt, MAGIC, None, op0=mybir.AluOpType.subtract
            )
            # whole-tile transpose on the DMA xbar: [tok, (kc ki)] -> [ki, kc, tok]
            nc.sync.dma_start(qT[:, t], q, transpose=True)

        # chunk 1 prefetch deferred to here so it doesn't delay the x loads
        if WC > 1:
            wtiles[1] = load_wchunk(1)

        # ---- xs broadcast tile [p, tok] = x_scale[tok] (via transpose+DMA) ----
        xs_rowT_ps = pt_pool.tile([TT, P], F32, tag="xs_t")
        nc.tensor.transpose(xs_rowT_ps, xs_all, ident_f32)
        xs_rowT = spool.tile([TT, P], F32, tag="xs_rowT")
        nc.vector.tensor_copy(xs_rowT, xs_rowT_ps)
        nc.sync.dma_start(xs_scratch.rearrange("(t p) -> t p", p=P), xs_rowT)
        xsb = spool.tile([P, S_C], F32, tag="xsb")
        nc.sync.dma_start(xsb, xs_scratch[None, :].to_broadcast((P, S_C)))

        # ---- Phase 2: streamed weights-stationary GEMM + fused dequant ----
        for wc in range(WC):
            wtile = wtiles.pop(wc) if wc in wtiles else load_wchunk(wc)
            for sub in range(NSUB):
                nt = wc * NSUB + sub
                ps = ps_pool.tile([P, S_C], F32)
                for kc in range(KC):
                    nc.tensor.matmul(
                        ps,
                        lhsT=wtile[:, kc, sub * P : (sub + 1) * P],
                        rhs=qT[:, :, kc, :],
                        start=(kc == 0),
                        stop=(kc == KC - 1),
                    )
                out_sb = opool.tile([P, S_C], F32)
                # out = (acc * w_scale[n]) * x_scale[tok]
                nc.vector.scalar_tensor_tensor(
                    out=out_sb,
                    in0=ps,
                    scalar=ws_sb[:, nt : nt + 1],
                    in1=xsb,
                    op0=mybir.AluOpType.mult,
                    op1=mybir.AluOpType.mult,
                )
                nc.sync.dma_start(outT_t[nt], out_sb)

    return nc


def pack_inputs(input_tensor, weight, weight_scale, S_C, K, N, NSUB=4):
    """Host-side prep: shard x, pack weights to bf16 SBUF-chunk layout."""
    KC = K // P
    NT = N // P
    WC = NT // NSUB
    x = np.ascontiguousarray(input_tensor.reshape(-1, K))  # [S, K]
    w_bf = weight.astype(ml_dtypes.bfloat16)  # [N, K], int8 values exact
    # pack[wc, p, kc, n] = w[wc*NSUB*P + n, kc*P + p]
    wt = np.ascontiguousarray(
        w_bf.reshape(WC, NSUB * P, KC, P).transpose(0, 3, 2, 1)
    )
    ws = np.ascontiguousarray(
        weight_scale.reshape(NT, P).T.astype(np.float32)
    )  # [P, NT]
    return x, wt, ws


@lru_cache(maxsize=2)
def _compiled_nc(S_C, K, N, NSUB, exact_divide):
    return build_nc(S_C, K, N, NSUB=NSUB, exact_divide=exact_divide)


def run(input_tensor, weight, weight_scale, n_cores=NCORES, trace=False,
        exact_divide=True, NSUB=4):
    Sfull, K = input_tensor.shape[-2], input_tensor.shape[-1]
    N = weight.shape[0]
    S_C = Sfull // n_cores
    x, wt, ws = pack_inputs(input_tensor, weight, weight_scale, S_C, K, N, NSUB)
    nc = _compiled_nc(S_C, K, N, NSUB, exact_divide)
    in_maps = [
        {"x": np.ascontiguousarray(x[c * S_C : (c + 1) * S_C]), "wt": wt, "ws": ws}
        for c in range(n_cores)
    ]
    res = run_bass_kernel_spmd(nc, in_maps, core_ids=list(range(n_cores)), trace=trace)
    out = np.empty((Sfull, N), np.float32)
    for c in range(n_cores):
        out[c * S_C : (c + 1) * S_C] = res.results[c]["outT"].T
    return out[None], res


def kernel(input_tensor, weight, weight_scale):
    out, _ = run(
        np.asarray(input_tensor), np.asarray(weight), np.asarray(weight_scale)
    )
    return out



# revision 10
# speedup vs baseline: 1.0078x; 1.0078x over previous
"""Int8 per-token-quantized linear (MluQuantLinearInt8) on 8 Trainium2 cores.

  out[s, n] = (sum_k q[s,k] * w[n,k]) * x_scale[s] * w_scale[n]
  q = round(x / x_scale) clipped to [-127, 127],  x_scale = max(|x|_row, 1e-8)/127

Sharding: data-parallel over tokens (512/core); weights replicated, streamed
once per core. Weights are host-pretransposed to [K, N] bf16 (int8 values are
exact in bf16, so int8xint8 products accumulate exactly in fp32 PSUM).

Schedule (v2): x tiles stream first so quantization starts immediately; the
per-tile quant chain is split into K-halves across vector/gpsimd/scalar; the
x_scale row broadcast is built on-chip (PE transpose + ones-matmul) instead of
a DRAM round trip; and weight-chunk 0's GEMM runs at 128-column granularity
per token tile as soon as that tile's qT lands (128-col matmuls measured at
full 0.44 ns/col cadence), with chunks 1..WC-1 in 512-column steady state.
"""

import sys
from contextlib import ExitStack
from functools import lru_cache

import numpy as np

for _p in ("/opt/trn_rl_repo", "/root/.axon_site/_ro/trn_rl_repo"):
    if _p not in sys.path:
        sys.path.append(_p)

import ml_dtypes  # noqa: E402

import concourse.bass as bass  # noqa: E402
import concourse.bass2jax as bass2jax  # noqa: E402
import concourse.mybir as mybir  # noqa: E402
import concourse.tile as tile  # noqa: E402
from concourse.bass_utils import (  # noqa: E402
    compile_bir_kernel as _orig_compile_bir_kernel,
    run_bass_kernel_spmd,
)
from concourse.masks import make_identity  # noqa: E402

# The walrus build in this container accepts only ONE sync-wait per
# instruction ("Too many sync wait commands", CoreV3GenImpl setupSyncWait) —
# Tile's kernel-tail drain carries several. Split extra waits onto preceding
# single-wait EventSemaphore carriers on the same engine (engine program order
# makes the AND of waits equivalent).
import json as _json  # noqa: E402


def _split_multi_waits(bir_json):
    d = _json.loads(bir_json)
    changed = False
    for fn in d.get("functions", []):
        for bb in fn.get("blocks", []) or []:
            insts = bb.get("instructions")
            if not insts:
                continue
            out = []
            for ins in insts:
                si = ins.get("sync_info")
                waits = (si or {}).get("on_wait") or []
                if len(waits) > 1:
                    for j, w in enumerate(waits[:-1]):
                        out.append(
                            {
                                "engine": ins.get("engine"),
                                "ins": [],
                                "outs": [],
                                "name": f"{ins.get('name', 'I')}_w{j}",
                                "opcode": "EventSemaphore",
                                "sync_info": {"on_update": [], "on_wait": [w]},
                            }
                        )
                    si["on_wait"] = [waits[-1]]
                    changed = True
                out.append(ins)
            bb["instructions"] = out
    if not changed:
        return bir_json
    return _json.dumps(d).encode()


def _patched_compile_bir_kernel(bir_json, tmpdir, neff_name="file.neff"):
    return _orig_compile_bir_kernel(
        _split_multi_waits(bir_json), tmpdir, neff_name=neff_name
    )


bass2jax.compile_bir_kernel = _patched_compile_bir_kernel

P = 128
NCORES = 8
S, K_FULL, N_FULL = 4096, 4096, 16384
QMAX = 127.0
MAGIC = 12582912.0  # 1.5 * 2**23: (y + MAGIC) - MAGIC == RNE-round(y) for |y| < 2**22
F32 = mybir.dt.float32
BF16 = mybir.dt.bfloat16


def build_nc(S_C, K, N, NSUB=4, exact_divide=True, early_start=True):
    """One-core program; SPMD-replicated across cores by the runner.

    Inputs (per core):
      x   [S_C, K]  f32 - this core's token slice
      wt  [WC, P, KC, NSUB*P] bf16 - weights, host-packed as SBUF-layout chunks
      ws  [P, NT]   f32 - weight_scale packed ws[p, nt] = weight_scale[nt*128+p]
    Output:
      outT [N, S_C] f32 - dequantized output, transposed
    """
    KC = K // P  # contraction chunks
    TT = S_C // P  # token tiles
    NT = N // P  # output-channel tiles (one psum tile each)
    WC = NT // NSUB  # streamed weight chunks
    KH = K // 2  # half the contraction dim (elements)
    KCH = KC // 2  # half the contraction dim (128-chunks)

    nc = bass.Bass()
    x = nc.declare_dram_parameter("x", [S_C, K], F32, isOutput=False)
    wt = nc.declare_dram_parameter("wt", [WC, P, KC, NSUB * P], BF16, isOutput=False)
    ws = nc.declare_dram_parameter("ws", [P, NT], F32, isOutput=False)
    outT = nc.declare_dram_parameter("outT", [N, S_C], F32, isOutput=True)

    outT_t = outT.rearrange("(nt p) s -> nt p s", p=P)

    with tile.TileContext(nc) as tc, ExitStack() as ctx:
        const_pool = ctx.enter_context(tc.tile_pool(name="const", bufs=1))
        xpool = ctx.enter_context(tc.tile_pool(name="xp", bufs=4))
        qpool = ctx.enter_context(tc.tile_pool(name="qp", bufs=2))
        qt_pool = ctx.enter_context(tc.tile_pool(name="qt", bufs=1))
        wpool = ctx.enter_context(tc.tile_pool(name="wp", bufs=2))
        opool = ctx.enter_context(tc.tile_pool(name="op", bufs=4))
        spool = ctx.enter_context(tc.tile_pool(name="sp", bufs=1))
        ps_pool = ctx.enter_context(tc.tile_pool(name="psp", bufs=7, space="PSUM"))
        xs_psp = ctx.enter_context(tc.tile_pool(name="xsps", bufs=1, space="PSUM"))

        # ---- DMA issue order: x tiles first (they pace everything), then the
        # small ws table, weight chunk 0, and chunk 1 after the x stream.
        xts = []
        for t in range(TT):
            xt = xpool.tile([P, K], F32, name="xt")
            nc.sync.dma_start(xt[:, :KH], x[t * P : (t + 1) * P, :KH])
            nc.sync.dma_start(xt[:, KH:], x[t * P : (t + 1) * P, KH:])
            xts.append(xt)
            if t == 0:
                ws_sb = const_pool.tile([P, NT], F32)
                nc.sync.dma_start(ws_sb, ws[:, :])

                def load_wchunk(wc):
                    wtile = wpool.tile([P, KC, NSUB * P], BF16, name="wtile")
                    nc.sync.dma_start(wtile[:, :KCH], wt[wc, :, :KCH])
                    nc.sync.dma_start(wtile[:, KCH:], wt[wc, :, KCH:])
                    return wtile

                wtiles = {0: load_wchunk(0)}

        ident_f32 = const_pool.tile([P, P], F32)
        make_identity(nc, ident_f32)
        ones_row = const_pool.tile([1, P], F32)
        nc.vector.memset(ones_row, 1.0)
        # preload the scalar engine's Copy activation table so tile 0's
        # quant doesn't pay the ~1.3us ACT_TABLE_LOAD on its critical path
        act_warm = const_pool.tile([1, P], F32)
        nc.scalar.activation(
            act_warm, ones_row, mybir.ActivationFunctionType.Copy,
            bias=0.0, scale=1.0,
        )

        qT = qt_pool.tile([P, TT, KC, P], BF16)
        xs_all = spool.tile([P, TT], F32)  # xs_all[p, t] = x_scale[t*128+p]
        xsb = spool.tile([P, S_C], F32)  # xsb[p, tok] = x_scale[tok]

        inv127 = float(np.float32(1.0 / 127.0))

        def quant_tile(t):
            """amax -> x_scale -> q (bf16) -> qT via XBAR, split in K-halves
            across vector/gpsimd/scalar so the chain latency is ~reduce+pre+sub."""
            xt = xts[t]
            amax_a = spool.tile([P, 1], F32, name="amax_a")
            amax_b = spool.tile([P, 1], F32, name="amax_b")
            # both on vector (gpsimd can't reduce along free axis); reduce_a
            # overlaps half-b's DMA, so amax lands ~2.3us after x_b arrives
            nc.vector.tensor_reduce(
                out=amax_a,
                in_=xt[:, :KH],
                axis=mybir.AxisListType.X,
                op=mybir.AluOpType.max,
                apply_absolute_value=True,
            )
            nc.vector.tensor_reduce(
                out=amax_b,
                in_=xt[:, KH:],
                axis=mybir.AxisListType.X,
                op=mybir.AluOpType.max,
                apply_absolute_value=True,
            )
            amax = spool.tile([P, 1], F32, name="amax")
            nc.vector.tensor_tensor(amax, amax_a, amax_b, op=mybir.AluOpType.max)
            nc.vector.tensor_scalar(amax, amax, 1e-8, None, op0=mybir.AluOpType.max)
            nc.vector.tensor_scalar(
                xs_all[:, t : t + 1], amax, inv127, None, op0=mybir.AluOpType.mult
            )
            inv = spool.tile([P, 1], F32, name="inv")
            nc.vector.reciprocal(inv, amax)
            nc.vector.tensor_scalar(inv, inv, QMAX, None, op0=mybir.AluOpType.mult)

            # x_scale broadcast for this tile's 128 tokens, all on-chip:
            # [P,1] column -> PE transpose -> [1,P] row -> ones-matmul -> [P,P]
            xs_ps = xs_psp.tile([P, P], F32, name="xs_ps")
            nc.tensor.transpose(xs_ps[0:1, :], xs_all[:, t : t + 1], ident_f32)
            xs_row = spool.tile([1, P], F32, name="xs_row")
            nc.vector.tensor_copy(xs_row, xs_ps[0:1, :])
            nc.tensor.matmul(xs_ps, lhsT=ones_row, rhs=xs_row, start=True, stop=True)
            nc.vector.tensor_copy(xsb[:, t * P : (t + 1) * P], xs_ps)

            # q = round(x * (127/amax)) via the +MAGIC/-MAGIC RNE trick.
            # Engine balance (gpsimd is ~17x too slow for bulk elementwise):
            # scalar: pre_a, pre_b, sub_b; vector: reduces, smalls, sub_a.
            # Half a's transpose (and the kc<KCH matmuls) start earliest.
            q = qpool.tile([P, K], BF16, name="q")
            nc.scalar.activation(
                xt[:, :KH], xt[:, :KH], mybir.ActivationFunctionType.Copy,
                bias=MAGIC, scale=inv,
            )
            nc.vector.tensor_scalar(
                q[:, :KH], xt[:, :KH], MAGIC, None, op0=mybir.AluOpType.subtract
            )
            nc.sync.dma_start(qT[:, t, :KCH], q[:, :KH], transpose=True)
            nc.scalar.activation(
                xt[:, KH:], xt[:, KH:], mybir.ActivationFunctionType.Copy,
                bias=MAGIC, scale=inv,
            )
            nc.scalar.activation(
                q[:, KH:], xt[:, KH:], mybir.ActivationFunctionType.Copy,
                bias=-MAGIC, scale=1.0,
            )
            nc.sync.dma_start(qT[:, t, KCH:], q[:, KH:], transpose=True)

        def dequant_store(ps, nt, c0, c1):
            """out[:, c0:c1] = (psum * w_scale[nt]) * x_scale[tok]; DMA out."""
            out_sb = opool.tile([P, S_C], F32, name="osb")
            nc.vector.scalar_tensor_tensor(
                out=out_sb[:, c0:c1],
                in0=ps,
                scalar=ws_sb[:, nt : nt + 1],
                in1=xsb[:, c0:c1],
                op0=mybir.AluOpType.mult,
                op1=mybir.AluOpType.mult,
            )
            nc.sync.dma_start(outT_t[nt][:, c0:c1], out_sb[:, c0:c1])

        if early_start:
            # Weight chunk 0 at 128-col granularity, per token tile as each
            # tile's qT lands (full matmul cadence measured at this width).
            for t in range(TT):
                quant_tile(t)
                w0 = wtiles[0]
                # two passes: all subs' kc<KCH first (only needs the half-a
                # transpose), then kc>=KCH once the half-b transpose landed
                pss = []
                for sub in range(NSUB):
                    ps = ps_pool.tile([P, S_C], F32, name="ps")
                    pss.append(ps)
                    for kc in range(KCH):
                        nc.tensor.matmul(
                            ps[:, :P],
                            lhsT=w0[:, kc, sub * P : (sub + 1) * P],
                            rhs=qT[:, t, kc, :],
                            start=(kc == 0),
                            stop=False,
                        )
                for sub in range(NSUB):
                    ps = pss[sub]
                    for kc in range(KCH, KC):
                        nc.tensor.matmul(
                            ps[:, :P],
                            lhsT=w0[:, kc, sub * P : (sub + 1) * P],
                            rhs=qT[:, t, kc, :],
                            start=False,
                            stop=(kc == KC - 1),
                        )
                    dequant_store(ps[:, :P], sub, t * P, (t + 1) * P)
                if t == 0:
                    # emitted here so the qT transposes (enqueued during
                    # quant_tile(0)) aren't stuck behind 4MB of weight DMA
                    wtiles[1] = load_wchunk(1)
            wtiles.pop(0)
            wc_start = 1
        else:
            for t in range(TT):
                quant_tile(t)
            wc_start = 0

        # ---- Steady state: streamed weights-stationary GEMM, 512-col ----
        for wc in range(wc_start, WC):
            wtile = wtiles.pop(wc) if wc in wtiles else load_wchunk(wc)
            for sub in range(NSUB):
                nt = wc * NSUB + sub
                ps = ps_pool.tile([P, S_C], F32, name="ps")
                for kc in range(KC):
                    nc.tensor.matmul(
                        ps,
                        lhsT=wtile[:, kc, sub * P : (sub + 1) * P],
                        rhs=qT[:, :, kc, :],
                        start=(kc == 0),
                        stop=(kc == KC - 1),
                    )
                dequant_store(ps, nt, 0, S_C)

    return nc


def pack_inputs(input_tensor, weight, weight_scale, S_C, K, N, NSUB=4):
    """Host-side prep: shard x, pack weights to bf16 SBUF-chunk layout."""
    KC = K // P
    NT = N // P
    WC = NT // NSUB
    x = np.ascontiguousarray(input_tensor.reshape(-1, K))  # [S, K]
    w_bf = weight.astype(ml_dtypes.bfloat16)  # [N, K], int8 values exact
    # pack[wc, p, kc, n] = w[wc*NSUB*P + n, kc*P + p]
    wt = np.ascontiguousarray(
        w_bf.reshape(WC, NSUB * P, KC, P).transpose(0, 3, 2, 1)
    )
    ws = np.ascontiguousarray(
        weight_scale.reshape(NT, P).T.astype(np.float32)
    )  # [P, NT]
    return x, wt, ws


@lru_cache(maxsize=2)
def _compiled_nc(S_C, K, N, NSUB, exact_divide):
    return build_nc(S_C, K, N, NSUB=NSUB, exact_divide=exact_divide)


def run(input_tensor, weight, weight_scale, n_cores=NCORES, trace=False,
        exact_divide=True, NSUB=4):
    Sfull, K = input_tensor.shape[-2], input_tensor.shape[-1]
    N = weight.shape[0]
    S_C = Sfull // n_cores
    x, wt, ws = pack_inputs(input_tensor, weight, weight_scale, S_C, K, N, NSUB)
    nc = _compiled_nc(S_C, K, N, NSUB, exact_divide)
    in_maps = [
        {"x": np.ascontiguousarray(x[c * S_C : (c + 1) * S_C]), "wt": wt, "ws": ws}
        for c in range(n_cores)
    ]
    res = run_bass_kernel_spmd(nc, in_maps, core_ids=list(range(n_cores)), trace=trace)
    out = np.empty((Sfull, N), np.float32)
    for c in range(n_cores):
        out[c * S_C : (c + 1) * S_C] = res.results[c]["outT"].T
    return out[None], res


def kernel(input_tensor, weight, weight_scale):
    out, _ = run(
        np.asarray(input_tensor), np.asarray(weight), np.asarray(weight_scale)
    )
    return out
